# revision 33
# baseline (speedup 1.0000x reference)
"""Trainium2 Bass kernel for the 2-layer LSTMCell model.

Model (per timestep t, torch.nn.LSTMCell semantics, gates (i,f,g,o)):
    h0,c0 = LSTMCell(x_t, (h0,c0))   # D_IN=16  -> H1=100
    h1,c1 = LSTMCell(h0, (h1,c1))    # H1=100 -> H2=50
    y = h1_final @ W_fc.T + b_fc     # [B, 1]

Strategy (8 NeuronCores, data parallel over batch, 256 rows/core):

  - Layer 0 runs fully TRANSPOSED: the recurrent state A(t) = [h0; 1; x_t]
    is [117, 256] (state dim on partitions), and the four gate matmuls use
    the constant weight blocks [117, 100] as stationary with A moving, so
    the gates land gate-major in PSUM and the whole c0/h0 update happens in
    that layout -- NO transposes anywhere on the layer-0 recurrence path.
    h0 is written by one DVE mult directly into the next step's A tile.
  - Matmul order f, g, i, o with one PSUM tile per sigma instruction, so
    sig(f) (needed first, for m2 = sig_f * C) never waits for later gates.
  - The g gate columns are host-doubled: tanh(g) = 2*sig(2g)-1 rides in the
    ordinary Sigmoid instructions, with the -0.5 / *2 fixups folded into
    scalar_tensor_tensor ops and a c/2-scaled cell state; tanh(c) is one
    real Tanh (same ACT table as Sigmoid) with scale=2.
  - Layer 1 is batch-major (gates [128, 2x200] from start/stop-adjacent
    accumulating matmul pairs), two steps behind layer 0; its h1 transpose
    + PSUM->SBUF copy are deferred to the NEXT iteration so they never sit
    not-ready at the head of the in-order PE/Vector queues.
  - nosync (ordering-only) edges keep the layer-1 sigmoid behind the
    chain-critical tanh(c0) on the ACT queue and the layer-1 matmul pairs
    out of the PE slot right after the h0 write: the greedy Tile scheduler
    would otherwise pick them and stall the recurrence chain.
  - x_t (plus the ones row) arrives via one small per-step DMA issued 3
    steps ahead, directly into the A tile -- nothing on the compute
    engines.
  - Known HW pitfalls baked in: PSUM accumulation pairs must be adjacent
    on the PE queue (interleaving corrupts results); GPSIMD cannot touch
    PSUM; scalar_tensor_tensor is Vector-only.

Measured: 8.53 ms on 8 cores (baseline this session: 9.8-10.2 ms), rel
err 2.3e-3.  The remaining period (~4.1 us/step) is the serial
c0-recurrence chain: MM -> sigmoid -> multiplies -> add -> tanh -> h0
write, each hop dominated by fixed instruction/access overheads.
"""

import sys

import ml_dtypes
import numpy as np

BF = ml_dtypes.bfloat16

sys.path.insert(0, "/opt/trn_rl_repo")

import concourse.bacc as bacc
import concourse.bass as bass
import concourse.mybir as mybir
from concourse.tile import TileContext

F32 = mybir.dt.float32
BF16 = mybir.dt.bfloat16
Act = mybir.ActivationFunctionType
Alu = mybir.AluOpType

B, T, D_IN = 2048, 2048, 16
H1, H2 = 100, 50
N_CORES = 8
B_LOCAL = B // N_CORES        # 256
NCH = 2                       # chunks of 128 per core

LAST_EXEC_NS = None

# ---------------------------------------------------------------- kernel build


def build_nc(t_steps=T):
    nc = bacc.Bacc("TRN2", target_bir_lowering=False)
    # x slots: row 0 = ones, rows 1:17 = x_t.T  -> A rows 100:117
    xt_d = nc.dram_tensor("xt", [t_steps + 4, 17, 256], BF16,
                          kind="ExternalInput").ap()
    cb_d = nc.dram_tensor("cblob", [128, 929], BF16, kind="ExternalInput").ap()
    a0_d = nc.dram_tensor("a0", [117, 256], BF16, kind="ExternalInput").ap()
    y_d = nc.dram_tensor("y", [1, 256], F32, kind="ExternalOutput").ap()

    LOOKAHEAD = 3

    with TileContext(nc) as tc:
        with (
            tc.tile_pool(name="consts", bufs=1) as cp,
            tc.tile_pool(name="apool", bufs=6) as apool,
            tc.tile_pool(name="bpool", bufs=3) as bpool,
            tc.tile_pool(name="ew", bufs=2) as ew,
            tc.tile_pool(name="g0ps", bufs=1, space="PSUM") as g0pool,
            tc.tile_pool(name="pops", bufs=1, space="PSUM") as popool,
            tc.tile_pool(name="g1ps", bufs=2, space="PSUM") as g1pool,
            tc.tile_pool(name="tps", bufs=1, space="PSUM") as tpool,
        ):
            cb = cp.tile([128, 929], BF16)
            nc.sync.dma_start(cb, cb_d)
            w0 = cb[0:117, 0:400]        # cols: g,i,f,o  (100 each)
            wih1 = cb[0:101, 400:600]    # cols: g,i,f,o  (50 each)
            whh1 = cb[0:50, 600:800]
            wfcb = cb[0:51, 800:801]     # rows 0:50 = W_fc, row 50 = b_fc
            ident = cb[0:128, 801:929]

            # A tiles: [117, 256]; rows 0:100 h0.T, row 100 ones, 101:117 x.T
            A_q = []

            def new_a(t):
                a = apool.tile([117, 256], BF16, tag="A")
                A_q.append(a)
                if t == 0:
                    nc.sync.dma_start(a, a0_d)
                else:
                    nc.sync.dma_start(a[100:117, :], xt_d[t])
                return a

            for k in range(LOOKAHEAD):
                new_a(k)

            BT = bpool.tile([50, 256], BF16, tag="B")
            nc.vector.memset(BT[:, :], 0.0)
            C = ew.tile([100, 256], BF16, tag="c0")
            nc.vector.memset(C[:, :], 0.0)
            C1 = ew.tile([128, 100], BF16, tag="c1")
            nc.vector.memset(C1[:, :], 0.0)

            state = {"BT": BT, "C1": C1, "after": None, "h1t": None,
                     "s1_pending": None}

            def l1_step(A):
                """One layer-1 step consuming h0 rows of A (+ ones row).

                The PSUM->SBUF transpose+copy of the PREVIOUS step's h1 is
                emitted here first: by now its inputs are long ready, so it
                never sits not-ready at the head of the PE/Vector queues."""
                l1_tail()
                BT, C1 = state["BT"], state["C1"]
                g1f = g1pool.tile([128, 512], F32, tag="g1")
                g1 = g1f[:, 0:400]
                for c in range(NCH):
                    mm_bi = nc.tensor.matmul(
                        g1[:, c * 200:c * 200 + 200],
                        A[0:101, c * 128:(c + 1) * 128],
                        wih1,
                        start=True, stop=False,
                    )
                    if c == 0 and state.get("po") is not None:
                        # keep the L1 pair-group out of the PE slot right
                        # after the h0 write (it would delay the next gate
                        # matmuls); it runs in the post-MM_o idle window
                        deps = bass.InstructionNameOrderedSet()
                        deps.add(state["po"])
                        mm_bi.ins.add_nosync_dependencies_from(deps)
                    nc.tensor.matmul(
                        g1[:, c * 200:c * 200 + 200],
                        BT[:, c * 128:(c + 1) * 128],
                        whh1,
                        start=False, stop=True,
                    )
                # one sigmoid over all four gates [sig2g | sig_i | sig_f | sig_o]
                S1 = ew.tile([128, 400], BF16, tag="S1")
                S1v = S1.rearrange("p (c g f) -> p c g f", c=2, g=4)
                g1v4 = g1.rearrange("p (c g f) -> p c g f", c=2, g=4)
                s1_bi = nc.scalar.activation(S1v, g1v4, Act.Sigmoid)
                state["s1_pending"] = s1_bi

                # m4' = (sig2g1 - 0.5)*sig_i1 = tanh(g1)*sig_i1 / 2
                m4 = ew.tile([128, 100], BF16, tag="m4")
                m4v = m4.rearrange("p (c f) -> p c f", c=2)
                nc.vector.scalar_tensor_tensor(
                    m4v, S1v[:, :, 0, :], 0.5, S1v[:, :, 1, :],
                    Alu.subtract, Alu.mult)
                m3 = ew.tile([128, 100], BF16, tag="m3")
                m3v = m3.rearrange("p (c f) -> p c f", c=2)
                C1v = C1.rearrange("p (c f) -> p c f", c=2)
                nc.gpsimd.tensor_tensor(m3v, S1v[:, :, 2, :], C1v, Alu.mult)
                C1n = ew.tile([128, 100], BF16, tag="c1")
                nc.gpsimd.tensor_tensor(C1n, m3, m4, Alu.add)
                thc1 = ew.tile([128, 100], BF16, tag="thc1")
                nc.scalar.activation(thc1, C1n, Act.Tanh, scale=2.0)
                h1t = ew.tile([128, 100], BF16, tag="h1t")
                h1tv = h1t.rearrange("p (c f) -> p c f", c=2)
                nc.vector.tensor_tensor(h1tv, S1v[:, :, 3, :], thc1.rearrange(
                    "p (c f) -> p c f", c=2), Alu.mult)
                state["C1"] = C1n
                state["h1t"] = h1t

            def l1_tail():
                """Transpose the stored h1t to [50, 256] and copy to SBUF."""
                h1t = state["h1t"]
                if h1t is None:
                    return
                ph1 = tpool.tile([50, 256], BF16, tag="ph1")
                for c in range(NCH):
                    nc.tensor.transpose(
                        ph1[:, c * 128:(c + 1) * 128],
                        h1t[:, c * 50:(c + 1) * 50],
                        ident,
                    )
                BTn = bpool.tile([50, 256], BF16, tag="B")
                nc.vector.tensor_copy(BTn, ph1)
                state["BT"] = BTn
                state["h1t"] = None

            for t in range(t_steps):
                A = A_q[t]
                An = A_q[t + 1]

                # ======== Layer 0, step t ========
                # MM order f, g, i, o -- sig_f lands first so m2 = sig_f*C
                # starts early; each sigma instruction has its own PSUM tile
                # so it waits only its own matmuls.
                pf = g0pool.tile([100, 256], F32, tag="pf")
                nc.tensor.matmul(pf, w0[:, 200:300], A, start=True, stop=True)
                pgi = g0pool.tile([100, 512], F32, tag="pgi")
                nc.tensor.matmul(pgi[:, 0:256], w0[:, 0:100], A,
                                 start=True, stop=True)
                nc.tensor.matmul(pgi[:, 256:512], w0[:, 100:200], A,
                                 start=True, stop=True)
                po = popool.tile([100, 256], F32, tag="po")
                po_bi = nc.tensor.matmul(po, w0[:, 300:400], A,
                                         start=True, stop=True)
                state["po"] = po_bi.ins.name

                # ======== Layer 1, step t-2 (two periods of slack) =======
                # emitted HERE so its matmuls land after this step's gate
                # matmuls on the in-order PE queue -- they then execute
                # during the sigmoid/tanh window instead of blocking the
                # next step's gate matmuls.
                if t > 1:
                    l1_step(A_q[t - 1])

                Sf = ew.tile([100, 256], BF16, tag="Sf")
                nc.scalar.activation(Sf, pf, Act.Sigmoid)
                Sgi = ew.tile([100, 512], BF16, tag="Sgi")
                nc.scalar.activation(Sgi, pgi, Act.Sigmoid)
                So = ew.tile([100, 256], BF16, tag="So")
                nc.scalar.activation(So, po, Act.Sigmoid)

                # m2' = sig_f * C   (C = c0/2, so Cn = m1' + m2' keeps c/2)
                m2 = ew.tile([100, 256], BF16, tag="m2")
                nc.vector.tensor_tensor(m2, Sf, C, Alu.mult)
                # m1' = (sig2g - 0.5)*sig_i = tanh(g)*sig_i / 2
                m1 = ew.tile([100, 256], BF16, tag="m1")
                nc.vector.scalar_tensor_tensor(
                    m1, Sgi[:, 0:256], 0.5, Sgi[:, 256:512],
                    Alu.subtract, Alu.mult)
                Cn = ew.tile([100, 256], BF16, tag="c0")
                nc.vector.tensor_tensor(Cn, m1, m2, Alu.add)
                # tanh(c0) = tanh(2*C) in one ACT instruction via scale
                thc = ew.tile([100, 256], BF16, tag="thc")
                with tc.high_priority():
                    thc_bi = nc.scalar.activation(thc, Cn, Act.Tanh, scale=2.0)
                    h0w_bi = nc.vector.tensor_tensor(
                        An[0:100, :], So, thc, Alu.mult)
                state["after"] = thc_bi.ins.name
                if state["s1_pending"] is not None:
                    # hard ordering: the layer-1 sigmoid must not jump ahead
                    # of this step's chain-critical tanh(c0) on the in-order
                    # ACT queue (greedy scheduler would otherwise pick it --
                    # it becomes ready a hair earlier every period)
                    deps = bass.InstructionNameOrderedSet()
                    deps.add(thc_bi.ins.name)
                    state["s1_pending"].ins.add_nosync_dependencies_from(deps)
                    state["s1_pending"] = None
                C = Cn

                # ======== prefetch A(t+LOOKAHEAD) ========
                if t + LOOKAHEAD <= t_steps:
                    new_a(t + LOOKAHEAD)

            # ---- epilogue: L1 steps for t_steps-2 and t_steps-1
            l1_step(A_q[t_steps - 1])
            l1_step(A_q[t_steps])
            l1_tail()
            # y = h1 @ W_fc.T + b_fc
            fin = ew.tile([51, 256], BF16, tag="fin")
            nc.vector.memset(fin[:, :], 1.0)
            nc.vector.tensor_copy(fin[0:50, :], state["BT"])
            yp = tpool.tile([1, 256], F32, tag="yp")
            nc.tensor.matmul(yp, wfcb, fin, start=True, stop=True)
            ysb = ew.tile([1, 256], F32, tag="ysb")
            nc.scalar.copy(ysb, yp)
            nc.sync.dma_start(y_d, ysb)
    return nc


# ---------------------------------------------------------------- host prep


def _pack_gates(w, h):
    """[4h, d] torch-order (i,f,g,o) -> [d, 4h] columns (2g, i, f, o).

    The g block is doubled so tanh(g) = 2*sigmoid(2g)-1 can ride in the
    same Sigmoid instruction as the other gates."""
    wt = np.asarray(w, np.float32).T if w.ndim == 2 else np.asarray(w, np.float32)[None, :]
    i, f, g, o = wt[:, 0:h], wt[:, h:2*h], wt[:, 2*h:3*h], wt[:, 3*h:4*h]
    return np.concatenate([2.0 * g, i, f, o], axis=1)


def prep_weights(W_ih0, W_hh0, b_ih0, b_hh0, W_ih1, W_hh1, b_ih1, b_hh1, W_fc, b_fc):
    cb = np.zeros((128, 929), np.float32)
    cb[0:100, 0:400] = _pack_gates(W_hh0, H1)
    cb[100, 0:400] = _pack_gates(np.asarray(b_ih0) + np.asarray(b_hh0), H1)[0]
    cb[101:117, 0:400] = _pack_gates(W_ih0, H1)
    cb[0:100, 400:600] = _pack_gates(W_ih1, H2)
    cb[100, 400:600] = _pack_gates(np.asarray(b_ih1) + np.asarray(b_hh1), H2)[0]
    cb[0:50, 600:800] = _pack_gates(W_hh1, H2)
    cb[0:50, 800] = np.asarray(W_fc, np.float32)[0]
    cb[50, 800] = float(np.asarray(b_fc).reshape(-1)[0])
    cb[:, 801:929] = np.eye(128, dtype=np.float32)
    return cb.astype(BF)


def prep_x_core(x_core, t_steps):
    """x_core [256, T, 16] -> bf16 [T+4, 17, 256]: row 0 ones, 1:17 x_t.T."""
    xt = np.zeros((t_steps + 4, 17, 256), BF)
    xt[:, 0, :] = np.asarray(1.0, BF)
    xt[:t_steps, 1:17, :] = (
        np.asarray(x_core, np.float32).transpose(1, 2, 0).astype(BF))
    return xt


_RUNNER_CACHE = {}


def _get_runner(t_steps):
    if t_steps in _RUNNER_CACHE:
        return _RUNNER_CACHE[t_steps]

    import jax
    from jax.experimental.shard_map import shard_map
    from jax.sharding import Mesh, NamedSharding, PartitionSpec

    from concourse import bass2jax

    bass2jax.install_neuronx_cc_hook()
    nc = build_nc(t_steps)
    if not nc.is_finalized():
        nc.finalize()
    global _LAST_NC
    _LAST_NC = nc

    partition_name = (
        nc.partition_id_tensor.name if nc.partition_id_tensor else None
    )
    in_names = []
    out_names = []
    out_avals = []
    zero_outs = []
    for alloc in nc.m.functions[0].allocations:
        if not isinstance(alloc, mybir.MemoryLocationSet):
            continue
        name = alloc.memorylocations[0].name
        if alloc.kind == "ExternalInput":
            if name == partition_name:
                continue
            in_names.append(name)
        elif alloc.kind == "ExternalOutput":
            out_names.append(name)
            shape = tuple(alloc.tensor_shape)
            dtype = mybir.dt.np(alloc.dtype)
            out_avals.append(jax.core.ShapedArray(shape, dtype))
            zero_outs.append(np.zeros(shape, dtype))
    n_params = len(in_names)
    all_in_names = in_names + out_names
    if partition_name is not None:
        all_in_names = all_in_names + [partition_name]

    def _body(*args):
        operands = list(args)
        if partition_name is not None:
            operands.append(bass2jax.partition_id_tensor())
        outs = bass2jax._bass_exec_p.bind(
            *operands,
            out_avals=tuple(out_avals),
            in_names=tuple(all_in_names),
            out_names=tuple(out_names),
            lowering_input_output_aliases=(),
            sim_require_finite=True,
            sim_require_nnan=True,
            nc=nc,
        )
        return tuple(outs)

    devices = jax.devices()[:N_CORES]
    mesh = Mesh(np.asarray(devices), ("core",))
    spec = PartitionSpec("core")
    in_specs = (spec,) * (n_params + len(out_names))
    out_specs = (spec,) * len(out_names)
    sharded = jax.jit(
        shard_map(_body, mesh=mesh, in_specs=in_specs, out_specs=out_specs,
                  check_rep=False),
        keep_unused=True,
    )
    sharding = NamedSharding(mesh, spec)

    def run(concat_inputs, n_bench=0):
        import time as _time

        args = [jax.device_put(concat_inputs[n], sharding) for n in in_names]
        args += [jax.device_put(
            np.zeros((N_CORES * z.shape[0], *z.shape[1:]), z.dtype), sharding)
            for z in zero_outs]
        outs = jax.block_until_ready(sharded(*args))
        bench_ns = None
        if n_bench:
            times = []
            for _ in range(n_bench):
                t0 = _time.perf_counter()
                jax.block_until_ready(sharded(*args))
                times.append(_time.perf_counter() - t0)
            bench_ns = int(min(times) * 1e9)
        y = np.asarray(outs[out_names.index("y")])
        return y, bench_ns

    _RUNNER_CACHE[t_steps] = run
    return run


def make_inputs(x, W_ih0, W_hh0, b_ih0, b_hh0, W_ih1, W_hh1, b_ih1, b_hh1,
                W_fc, b_fc):
    x = np.asarray(x, dtype=np.float32)
    t_steps = x.shape[1]
    cb = prep_weights(
        W_ih0, W_hh0, b_ih0, b_hh0, W_ih1, W_hh1, b_ih1, b_hh1, W_fc, b_fc
    )
    nslot = t_steps + 4
    xt_all = np.empty((N_CORES * nslot, 17, 256), BF)
    a0_all = np.zeros((N_CORES * 117, 256), BF)
    for core in range(N_CORES):
        xc = x[core * B_LOCAL:(core + 1) * B_LOCAL]
        xt = prep_x_core(xc, t_steps)
        xt_all[core * nslot:(core + 1) * nslot] = xt
        a0_all[core * 117 + 100:(core + 1) * 117] = xt[0]
    reps = lambda a: np.concatenate([a] * N_CORES, axis=0)
    return t_steps, {
        "xt": xt_all,
        "cblob": reps(cb),
        "a0": a0_all,
    }


def kernel(x, W_ih0, W_hh0, b_ih0, b_hh0, W_ih1, W_hh1, b_ih1, b_hh1, W_fc, b_fc,
           n_bench=0):
    global LAST_EXEC_NS
    t_steps, concat_inputs = make_inputs(
        x, W_ih0, W_hh0, b_ih0, b_hh0, W_ih1, W_hh1, b_ih1, b_hh1, W_fc, b_fc
    )
    run = _get_runner(t_steps)
    y, bench_ns = run(concat_inputs, n_bench=n_bench)
    if bench_ns is not None:
        LAST_EXEC_NS = bench_ns
    return np.ascontiguousarray(y.reshape(-1)[:, None]).astype(np.float32)
